# revision 9
# baseline (speedup 1.0000x reference)
"""3x3 neighborhood (ADDA) attention on Trainium2, B=8, d=512 (8 heads x 64), 56x56.

Sharding: pure data parallel per the hint — batch b -> NeuronCore b (8 cores,
SPMD, no cross-core communication). Each core computes full local attention for
one batch.

Device design (per core, 4 head-pair groups):
  All compute ops run on partitions [0:116) = 2 heads x 58 padded rows (HW
  requires >64-partition accesses to start at partition 0, so the window's row
  shift dy is baked into host-prepared zero-padded images; OOB window slots
  contribute logit 0 / value 0, matching torch-Unfold zero-pad semantics, and
  softmax runs over all 9 slots).

  QK stage uses channel-INNER tiles [116, 60x, 64c]: 9 DVE bf16 mults (2x
  mode) + contiguous channel tensor_reduce into L[:, j, :]. Softmax without
  max subtraction (logits are O(5)) with the 1/sqrt(64) scale folded into the
  ACT exp. AV stage uses channel-OUTER tiles [116, 64c, 60x] so the per-pixel
  weight broadcasts along the outer free dim and mults stay in 2x mode (v has
  two x-parity variants to keep odd dx 4B-aligned). The 9 AV products are
  summed on the otherwise idle TensorEngine: identity-weight matmuls
  accumulating into PSUM f32; ACT casts PSUM -> bf16 output.

  DMA: descriptor GENERATION on the issuing sequencer is the scaling limit
  (~one descriptor per partition row per DMA), so each tensor family arrives
  as ONE DMA of a host-concatenated variant image (k: 3 dy variants, v: 6
  dy/x-parity variants, contiguous per partition), issued on the GPSIMD SWDGE
  (parallel descriptor generation) while the output store uses the SP HWDGE.
"""
import sys

sys.path.insert(0, "/opt/trn_rl_repo")

from contextlib import ExitStack

import ml_dtypes
import numpy as np

import concourse.bacc as bacc
import concourse.tile as tile
from concourse import mybir
from concourse.bass_utils import run_bass_kernel_spmd

B, D, H, W = 8, 512, 56, 56
NH, HD = 8, 64
SCALE = HD ** (-0.5)
N_CORES = 8
NG = 4          # head-pair groups
P58 = 58        # tile rows per head (1 + 56 + 1)
NP = 116        # compute partitions (2 heads x 58)
XT = 60         # padded x extent (even -> interiors stay 4B-aligned in bf16)
XI = 2          # interior x start
FLAT = HD * W   # 3584
BF16 = mybir.dt.bfloat16
F32 = mybir.dt.float32
BF = ml_dtypes.bfloat16

OFFS = [(dy, dx, dx & 1) for dy in (-1, 0, 1) for dx in (-1, 0, 1)]
VVARS = [(dy, xp) for dy in (-1, 0, 1) for xp in (0, 1)]

_NC_CACHE = {}


def _build_program():
    nc = bacc.Bacc("TRN2", target_bir_lowering=False, debug=False,
                   num_devices=N_CORES)
    q_d = nc.declare_dram_parameter("q", [NG, NP, XT, HD], BF16, isOutput=False)
    k_d = nc.declare_dram_parameter("k", [NG, NP, 3, XT, HD], BF16,
                                    isOutput=False)
    v_d = nc.declare_dram_parameter("v", [NG, NP, 6, HD, XT], BF16,
                                    isOutput=False)
    i_d = nc.declare_dram_parameter("ident", [NP, NP], BF16, isOutput=False)
    o_d = nc.declare_dram_parameter("out", [NG, NP, HD, W], BF16, isOutput=True)

    with tile.TileContext(nc) as tc:
        with ExitStack() as ctx:
            one_pool = ctx.enter_context(tc.tile_pool(name="one", bufs=1))
            q_pool = ctx.enter_context(tc.tile_pool(name="q", bufs=1))
            io_pool = ctx.enter_context(tc.tile_pool(name="io", bufs=2))
            tmp_pool = ctx.enter_context(tc.tile_pool(name="tmp", bufs=2))
            sm_pool = ctx.enter_context(tc.tile_pool(name="sm", bufs=2))
            ob_pool = ctx.enter_context(tc.tile_pool(name="ob", bufs=1))
            ps_pool = ctx.enter_context(
                tc.tile_pool(name="ps", bufs=1, space="PSUM"))

            ident = one_pool.tile([NP, NP], BF16)
            nc.sync.dma_start(out=ident[:], in_=i_d[:])

            for g in range(NG):
                qt = q_pool.tile([NP, XT, HD], BF16, tag="qt")
                nc.sync.dma_start(out=qt[:], in_=q_d[g])
                # k family: 3 dy-shifted variant images in one DMA
                ka = io_pool.tile([NP, 3, XT, HD], BF16, tag="ka")
                nc.scalar.dma_start(out=ka[:], in_=k_d[g])

                L = sm_pool.tile([NP, 9, W], F32, tag="L")
                Pt = sm_pool.tile([NP, 9, W], BF16, tag="P")
                Wt = sm_pool.tile([NP, 9, W], BF16, tag="W")
                S = sm_pool.tile([NP, W], F32, tag="S")
                R = sm_pool.tile([NP, W], F32, tag="R")

                # --- QK: logits (channel-inner), 2x-mode pairwise tree sum ---
                for j, (dy, dx, xp) in enumerate(OFFS):
                    tm = tmp_pool.tile([NP, W, HD], BF16, tag="tm")
                    nc.vector.tensor_mul(
                        tm[:, :, :],
                        qt[:, XI:XI + W, :],
                        ka[:, dy + 1, XI + dx:XI + dx + W, :],
                    )
                    t32 = tmp_pool.tile([NP, W, 32], BF16, tag="t32")
                    nc.vector.tensor_add(t32[:], tm[:, :, 0:32],
                                         tm[:, :, 32:64])
                    t16 = tmp_pool.tile([NP, W, 16], BF16, tag="t16")
                    nc.vector.tensor_add(t16[:], t32[:, :, 0:16],
                                         t32[:, :, 16:32])
                    t8 = tmp_pool.tile([NP, W, 8], BF16, tag="t8")
                    nc.vector.tensor_add(t8[:], t16[:, :, 0:8],
                                         t16[:, :, 8:16])
                    t4 = tmp_pool.tile([NP, W, 4], BF16, tag="t4")
                    nc.vector.tensor_add(t4[:], t8[:, :, 0:4], t8[:, :, 4:8])
                    t2 = tmp_pool.tile([NP, W, 2], F32, tag="t2")
                    nc.vector.tensor_add(t2[:], t4[:, :, 0:2], t4[:, :, 2:4])
                    nc.vector.tensor_add(L[:, j, :], t2[:, :, 0], t2[:, :, 1])

                # --- softmax (no max subtraction; SCALE folded into exp) ---
                nc.scalar.activation(
                    out=Pt[:, :, :], in_=L[:, :, :],
                    func=mybir.ActivationFunctionType.Exp, scale=float(SCALE),
                )
                nc.vector.tensor_reduce(
                    out=S[:, :],
                    in_=Pt[:, :, :].transpose([0, 2, 1]),
                    axis=mybir.AxisListType.X,
                    op=mybir.AluOpType.add,
                )
                nc.vector.reciprocal(out=R[:, :], in_=S[:, :])
                nc.vector.tensor_mul(
                    Wt[:, :, :],
                    Pt[:, :, :],
                    R[:, :].unsqueeze(1).to_broadcast((NP, 9, W)),
                )

                # --- AV: products (channel-outer) + PE identity-accumulate ---
                va = io_pool.tile([NP, 6, HD, XT], BF16, tag="va")
                nc.gpsimd.dma_start(out=va[:], in_=v_d[g])

                av = ps_pool.tile([NP, FLAT], F32, tag="av")
                for j, (dy, dx, xp) in enumerate(OFFS):
                    ta = tmp_pool.tile([NP, HD, W], BF16, tag="ta")
                    xb = XI + xp + dx
                    eng = nc.gpsimd if j >= 7 else nc.vector
                    eng.tensor_mul(
                        ta[:, :, :],
                        Wt[:, j:j + 1, :].to_broadcast((NP, HD, W)),
                        va[:, VVARS.index((dy, xp)), :, xb:xb + W],
                    )
                    taf = ta[:, :, :].rearrange("p c x -> p (c x)")
                    for ch in range(FLAT // 512):
                        nc.tensor.matmul(
                            av[:, ch * 512:(ch + 1) * 512],
                            ident[:],
                            taf[:, ch * 512:(ch + 1) * 512],
                            start=(j == 0),
                            stop=(j == 8),
                        )

                ob = ob_pool.tile([NP, HD, W], BF16, tag="ob")
                nc.scalar.copy(ob[:, :, :], av[:, :].rearrange(
                    "p (c x) -> p c x", c=HD))
                nc.sync.dma_start(out=o_d[g], in_=ob[:])

    nc.compile()
    return nc


def _get_nc():
    if "nc" not in _NC_CACHE:
        _NC_CACHE["nc"] = _build_program()
    return _NC_CACHE["nc"]


def _prep_inputs(q, k, v):
    """Build per-core variant images (leading dim = core/batch).

    q: [B, NG, 116, 60, 64]    k: [B, NG, 116, 3, 60, 64] (dy in {-1,0,1})
    v: [B, NG, 116, 6, 64, 60] ((dy, xp) in VVARS order)
    Tile row p = hh*58 + pr holds image row y = pr - 1 (+dy for variants);
    out-of-range rows and x pads are zero.
    """
    qyxc = q.reshape(B, NH, HD, H, W).transpose(0, 1, 3, 4, 2).astype(BF)
    kyxc = k.reshape(B, NH, HD, H, W).transpose(0, 1, 3, 4, 2).astype(BF)
    vycx = v.reshape(B, NH, HD, H, W).transpose(0, 1, 3, 2, 4).astype(BF)

    qi = np.zeros((B, NG, NP, XT, HD), dtype=BF)
    ki = np.zeros((B, NG, NP, 3, XT, HD), dtype=BF)
    vi = np.zeros((B, NG, NP, 6, HD, XT), dtype=BF)
    for g in range(NG):
        for hh in range(2):
            hd = 2 * g + hh
            p0 = hh * P58
            qi[:, g, p0 + 1:p0 + 1 + H, XI:XI + W, :] = qyxc[:, hd]
            for di, dy in enumerate((-1, 0, 1)):
                a, b = max(0, 1 - dy), min(P58, P58 - 1 - dy)
                ki[:, g, p0 + a:p0 + b, di, XI:XI + W, :] = \
                    kyxc[:, hd, a - 1 + dy:b - 1 + dy]
                for xp in (0, 1):
                    vi[:, g, p0 + a:p0 + b, VVARS.index((dy, xp)), :,
                       XI + xp:XI + xp + W] = vycx[:, hd, a - 1 + dy:b - 1 + dy]
    ident = np.eye(NP, dtype=BF)
    return [{"q": qi[b], "k": ki[b], "v": vi[b], "ident": ident}
            for b in range(N_CORES)]


def _run(q, k, v, trace=False, tmpdir=None):
    q = np.asarray(q, dtype=np.float32)
    k = np.asarray(k, dtype=np.float32)
    v = np.asarray(v, dtype=np.float32)
    in_maps = _prep_inputs(q, k, v)
    nc = _get_nc()
    res = run_bass_kernel_spmd(nc, in_maps, core_ids=list(range(N_CORES)),
                               trace=trace, tmpdir=tmpdir)
    # out image [NG, 116, 64, 56] -> [y, x, c]
    out = np.empty((B, H, W, D), dtype=np.float32)
    for b in range(N_CORES):
        oi = np.asarray(res.results[b]["out"]).astype(np.float32)
        for g in range(NG):
            for hh in range(2):
                hd = 2 * g + hh
                blk = oi[g, hh * P58 + 1:hh * P58 + 1 + H]     # [y, c, x]
                out[b, :, :, hd * HD:(hd + 1) * HD] = blk.transpose(0, 2, 1)
    return out, res


def kernel(q, k, v):
    out, _ = _run(q, k, v, trace=False)
    return out


def run_traced(q, k, v, tmpdir=None):
    out, res = _run(q, k, v, trace=True, tmpdir=tmpdir)
    return out, res


# revision 10
# speedup vs baseline: 1.0966x; 1.0966x over previous
"""3x3 neighborhood (ADDA) attention on Trainium2, B=8, d=512 (8 heads x 64), 56x56.

Sharding: pure data parallel per the hint — batch b -> NeuronCore b (8 cores,
SPMD, no cross-core communication). Each core computes full local attention for
one batch.

Device design (per core, 4 head-pair groups):
  All compute ops run on partitions [0:116) = 2 heads x 58 padded rows (HW
  requires >64-partition accesses to start at partition 0, so the window's row
  shift dy is baked into host-prepared zero-padded images; OOB window slots
  contribute logit 0 / value 0, matching torch-Unfold zero-pad semantics, and
  softmax runs over all 9 slots).

  QK stage uses channel-INNER tiles [116, 60x, 64c]: 9 DVE bf16 mults (2x
  mode) + contiguous channel tensor_reduce into L[:, j, :]. Softmax without
  max subtraction (logits are O(5)) with the 1/sqrt(64) scale folded into the
  ACT exp. AV stage uses channel-OUTER tiles [116, 64c, 60x] so the per-pixel
  weight broadcasts along the outer free dim and mults stay in 2x mode (v has
  two x-parity variants to keep odd dx 4B-aligned). The 9 AV products are
  summed on the otherwise idle TensorEngine: identity-weight matmuls
  accumulating into PSUM f32; ACT casts PSUM -> bf16 output.

  DMA: descriptor GENERATION on the issuing sequencer is the scaling limit
  (~one descriptor per partition row per DMA), so each tensor family arrives
  as ONE DMA of a host-concatenated variant image (k: 3 dy variants, v: 6
  dy/x-parity variants, contiguous per partition), issued on the GPSIMD SWDGE
  (parallel descriptor generation) while the output store uses the SP HWDGE.
"""
import sys

sys.path.insert(0, "/opt/trn_rl_repo")

from contextlib import ExitStack

import ml_dtypes
import numpy as np

import concourse.bacc as bacc
import concourse.tile as tile
from concourse import mybir
from concourse.bass_utils import run_bass_kernel_spmd

B, D, H, W = 8, 512, 56, 56
NH, HD = 8, 64
SCALE = HD ** (-0.5)
N_CORES = 8
NG = 4          # head-pair groups
P58 = 58        # tile rows per head (1 + 56 + 1)
NP = 116        # compute partitions (2 heads x 58)
XT = 60         # padded x extent (even -> interiors stay 4B-aligned in bf16)
XI = 2          # interior x start
FLAT = HD * W   # 3584
BF16 = mybir.dt.bfloat16
F32 = mybir.dt.float32
BF = ml_dtypes.bfloat16

OFFS = [(dy, dx, dx & 1) for dy in (-1, 0, 1) for dx in (-1, 0, 1)]
VVARS = [(dy, xp) for dy in (-1, 0, 1) for xp in (0, 1)]

_NC_CACHE = {}


def _build_program():
    nc = bacc.Bacc("TRN2", target_bir_lowering=False, debug=False,
                   num_devices=N_CORES)
    q_d = nc.declare_dram_parameter("q", [NG, NP, XT, HD], BF16, isOutput=False)
    k_d = nc.declare_dram_parameter("k", [NG, NP, 3, XT, HD], BF16,
                                    isOutput=False)
    v_d = nc.declare_dram_parameter("v", [NG, NP, 6, HD, XT], BF16,
                                    isOutput=False)
    i_d = nc.declare_dram_parameter("ident", [NP, NP], BF16, isOutput=False)
    o_d = nc.declare_dram_parameter("out", [NG, NP, HD, W], BF16, isOutput=True)

    with tile.TileContext(nc) as tc:
        with ExitStack() as ctx:
            one_pool = ctx.enter_context(tc.tile_pool(name="one", bufs=1))
            q_pool = ctx.enter_context(tc.tile_pool(name="q", bufs=1))
            io_pool = ctx.enter_context(tc.tile_pool(name="io", bufs=2))
            tmp_pool = ctx.enter_context(tc.tile_pool(name="tmp", bufs=2))
            sm_pool = ctx.enter_context(tc.tile_pool(name="sm", bufs=2))
            ob_pool = ctx.enter_context(tc.tile_pool(name="ob", bufs=1))
            ps_pool = ctx.enter_context(
                tc.tile_pool(name="ps", bufs=1, space="PSUM"))

            ident = one_pool.tile([NP, NP], BF16)
            nc.sync.dma_start(out=ident[:], in_=i_d[:])

            for g in range(NG):
                qt = q_pool.tile([NP, XT, HD], BF16, tag="qt")
                nc.sync.dma_start(out=qt[:], in_=q_d[g])
                # k family: 3 dy-shifted variant images in one DMA
                ka = io_pool.tile([NP, 3, XT, HD], BF16, tag="ka")
                nc.scalar.dma_start(out=ka[:], in_=k_d[g])

                L = sm_pool.tile([NP, 9, W], F32, tag="L")
                Pt = sm_pool.tile([NP, 9, W], BF16, tag="P")
                Wt = sm_pool.tile([NP, 9, W], BF16, tag="W")
                S = sm_pool.tile([NP, W], F32, tag="S")
                R = sm_pool.tile([NP, W], F32, tag="R")

                # --- QK: logits (channel-inner), 2x-mode pairwise tree sum ---
                for j, (dy, dx, xp) in enumerate(OFFS):
                    tm = tmp_pool.tile([NP, W, HD], BF16, tag="tm")
                    nc.vector.tensor_mul(
                        tm[:, :, :],
                        qt[:, XI:XI + W, :],
                        ka[:, dy + 1, XI + dx:XI + dx + W, :],
                    )
                    t32 = tmp_pool.tile([NP, W, 32], BF16, tag="t32")
                    nc.vector.tensor_add(t32[:], tm[:, :, 0:32],
                                         tm[:, :, 32:64])
                    t16 = tmp_pool.tile([NP, W, 16], BF16, tag="t16")
                    nc.vector.tensor_add(t16[:], t32[:, :, 0:16],
                                         t32[:, :, 16:32])
                    t8 = tmp_pool.tile([NP, W, 8], BF16, tag="t8")
                    nc.vector.tensor_add(t8[:], t16[:, :, 0:8],
                                         t16[:, :, 8:16])
                    t4 = tmp_pool.tile([NP, W, 4], BF16, tag="t4")
                    nc.vector.tensor_add(t4[:], t8[:, :, 0:4], t8[:, :, 4:8])
                    t2 = tmp_pool.tile([NP, W, 2], F32, tag="t2")
                    nc.vector.tensor_add(t2[:], t4[:, :, 0:2], t4[:, :, 2:4])
                    nc.vector.tensor_add(L[:, j, :], t2[:, :, 0], t2[:, :, 1])

                # --- softmax (no max subtraction; SCALE folded into exp) ---
                nc.scalar.activation(
                    out=Pt[:, :, :], in_=L[:, :, :],
                    func=mybir.ActivationFunctionType.Exp, scale=float(SCALE),
                )
                nc.vector.tensor_reduce(
                    out=S[:, :],
                    in_=Pt[:, :, :].transpose([0, 2, 1]),
                    axis=mybir.AxisListType.X,
                    op=mybir.AluOpType.add,
                )
                nc.vector.reciprocal(out=R[:, :], in_=S[:, :])
                nc.vector.tensor_mul(
                    Wt[:, :, :],
                    Pt[:, :, :],
                    R[:, :].unsqueeze(1).to_broadcast((NP, 9, W)),
                )

                # --- AV: products (channel-outer) + PE identity-accumulate ---
                va = io_pool.tile([NP, 6, HD, XT], BF16, tag="va")
                nc.gpsimd.dma_start(out=va[:], in_=v_d[g])

                av = ps_pool.tile([NP, FLAT], F32, tag="av")
                for j, (dy, dx, xp) in enumerate(OFFS):
                    ta = tmp_pool.tile([NP, HD, W], BF16, tag="ta")
                    xb = XI + xp + dx
                    nc.vector.tensor_mul(
                        ta[:, :, :],
                        Wt[:, j:j + 1, :].to_broadcast((NP, HD, W)),
                        va[:, VVARS.index((dy, xp)), :, xb:xb + W],
                    )
                    taf = ta[:, :, :].rearrange("p c x -> p (c x)")
                    for ch in range(FLAT // 512):
                        nc.tensor.matmul(
                            av[:, ch * 512:(ch + 1) * 512],
                            ident[:],
                            taf[:, ch * 512:(ch + 1) * 512],
                            start=(j == 0),
                            stop=(j == 8),
                        )

                ob = ob_pool.tile([NP, HD, W], BF16, tag="ob")
                nc.scalar.copy(ob[:, :, :], av[:, :].rearrange(
                    "p (c x) -> p c x", c=HD))
                nc.sync.dma_start(out=o_d[g], in_=ob[:])

    nc.compile()
    return nc


def _get_nc():
    if "nc" not in _NC_CACHE:
        _NC_CACHE["nc"] = _build_program()
    return _NC_CACHE["nc"]


def _prep_inputs(q, k, v):
    """Build per-core variant images (leading dim = core/batch).

    q: [B, NG, 116, 60, 64]    k: [B, NG, 116, 3, 60, 64] (dy in {-1,0,1})
    v: [B, NG, 116, 6, 64, 60] ((dy, xp) in VVARS order)
    Tile row p = hh*58 + pr holds image row y = pr - 1 (+dy for variants);
    out-of-range rows and x pads are zero.
    """
    qyxc = q.reshape(B, NH, HD, H, W).transpose(0, 1, 3, 4, 2).astype(BF)
    kyxc = k.reshape(B, NH, HD, H, W).transpose(0, 1, 3, 4, 2).astype(BF)
    vycx = v.reshape(B, NH, HD, H, W).transpose(0, 1, 3, 2, 4).astype(BF)

    qi = np.zeros((B, NG, NP, XT, HD), dtype=BF)
    ki = np.zeros((B, NG, NP, 3, XT, HD), dtype=BF)
    vi = np.zeros((B, NG, NP, 6, HD, XT), dtype=BF)
    for g in range(NG):
        for hh in range(2):
            hd = 2 * g + hh
            p0 = hh * P58
            qi[:, g, p0 + 1:p0 + 1 + H, XI:XI + W, :] = qyxc[:, hd]
            for di, dy in enumerate((-1, 0, 1)):
                a, b = max(0, 1 - dy), min(P58, P58 - 1 - dy)
                ki[:, g, p0 + a:p0 + b, di, XI:XI + W, :] = \
                    kyxc[:, hd, a - 1 + dy:b - 1 + dy]
                for xp in (0, 1):
                    vi[:, g, p0 + a:p0 + b, VVARS.index((dy, xp)), :,
                       XI + xp:XI + xp + W] = vycx[:, hd, a - 1 + dy:b - 1 + dy]
    ident = np.eye(NP, dtype=BF)
    return [{"q": qi[b], "k": ki[b], "v": vi[b], "ident": ident}
            for b in range(N_CORES)]


def _run(q, k, v, trace=False, tmpdir=None):
    q = np.asarray(q, dtype=np.float32)
    k = np.asarray(k, dtype=np.float32)
    v = np.asarray(v, dtype=np.float32)
    in_maps = _prep_inputs(q, k, v)
    nc = _get_nc()
    res = run_bass_kernel_spmd(nc, in_maps, core_ids=list(range(N_CORES)),
                               trace=trace, tmpdir=tmpdir)
    # out image [NG, 116, 64, 56] -> [y, x, c]
    out = np.empty((B, H, W, D), dtype=np.float32)
    for b in range(N_CORES):
        oi = np.asarray(res.results[b]["out"]).astype(np.float32)
        for g in range(NG):
            for hh in range(2):
                hd = 2 * g + hh
                blk = oi[g, hh * P58 + 1:hh * P58 + 1 + H]     # [y, c, x]
                out[b, :, :, hd * HD:(hd + 1) * HD] = blk.transpose(0, 2, 1)
    return out, res


def kernel(q, k, v):
    out, _ = _run(q, k, v, trace=False)
    return out


def run_traced(q, k, v, tmpdir=None):
    out, res = _run(q, k, v, trace=True, tmpdir=tmpdir)
    return out, res
